# revision 14
# baseline (speedup 1.0000x reference)
"""Trainium2 Bass kernel for nn_AttentionNet (additive attention + masked softmax).

Math (per batch b):
    D[h, u] = sum_k Wu_eff[k, h] * userT[k, u] + btot[h]      (Wu_eff = Wu @ W2, btot = bu@W2 + bs@W1)
    E[h, s] = sum_k Ws_eff[k, h] * servT[k, s]                (Ws_eff = Ws[:6] @ W1)
    u_i[u, s] = sum_h vt[h] * tanh(E[h, s] + D[h, u])
    probs[u, :] = softmax(10 * where(mask, u_i, log(1e-45)))

Device mapping (8 cores, 2 batches each):
    - DVE: per-u bias-add X[:, u-slice] = E + D[:, u]   (fp32 tensor_scalar, 2x mode)
    - ACT: tanh over [128, G*256] blocks (fp32 -> fp16), exp for softmax
    - PE : vt-dot as M=32 matmuls with a sliding-window weight (vt at one
           column of a zero [128, 64] tile), accumulating rows into a
           [128, 512] PSUM tile holding 2 u per row (u on partitions)
    - softmax over s entirely in the free dimension; masked entries are
      exactly 0 (exp underflow in the reference), applied as a multiply.
"""

import numpy as np
from contextlib import ExitStack

import concourse.bass as bass
import concourse.bacc as bacc
import concourse.mybir as mybir
import concourse.tile as tile
from concourse.bass_utils import run_bass_kernel_spmd

F32 = mybir.dt.float32
F16 = mybir.dt.float16
U8 = mybir.dt.uint8
AF = mybir.ActivationFunctionType
AX = mybir.AxisListType

N_CORES = 8
B, U, S, H = 16, 500, 256, 128
BC = B // N_CORES  # batches per core
G = 64             # user-steps per tanh block

_CACHE = {}


def _build_nc():
    nc = bacc.Bacc("TRN2", target_bir_lowering=False, debug=False)
    userT = nc.dram_tensor("userT", [BC, 3, U], F32, kind="ExternalInput")
    servT = nc.dram_tensor("servT", [BC, 6, S], F32, kind="ExternalInput")
    masks = nc.dram_tensor("masks", [BC, U, S], U8, kind="ExternalInput")
    wu = nc.dram_tensor("wu_eff", [3, H], F32, kind="ExternalInput")
    ws = nc.dram_tensor("ws_eff", [6, H], F32, kind="ExternalInput")
    btot = nc.dram_tensor("btot", [H, 1], F32, kind="ExternalInput")
    vt = nc.dram_tensor("vt", [H, 1], F32, kind="ExternalInput")
    out = nc.dram_tensor("probs", [BC, U, S], F32, kind="ExternalOutput")

    with ExitStack() as ctx:
        tc = ctx.enter_context(tile.TileContext(nc))
        const = ctx.enter_context(tc.tile_pool(name="const", bufs=1))
        pre = ctx.enter_context(tc.tile_pool(name="pre", bufs=2))
        dpool = ctx.enter_context(tc.tile_pool(name="dp", bufs=2))
        epool = ctx.enter_context(tc.tile_pool(name="ep", bufs=2))
        xpool = ctx.enter_context(tc.tile_pool(name="xp", bufs=2))
        tpool = ctx.enter_context(tc.tile_pool(name="tp", bufs=2))
        mpool = ctx.enter_context(tc.tile_pool(name="mp", bufs=2))
        sxpool = ctx.enter_context(tc.tile_pool(name="sx", bufs=2))
        stpool = ctx.enter_context(tc.tile_pool(name="st", bufs=4))
        prpool = ctx.enter_context(tc.tile_pool(name="pp", bufs=2))
        pps = ctx.enter_context(tc.tile_pool(name="pps", bufs=1, space="PSUM"))
        mps = ctx.enter_context(tc.tile_pool(name="mps", bufs=4, space="PSUM"))

        wu_sb = const.tile([3, H], F32)
        nc.sync.dma_start(wu_sb[:], wu[:])
        ws_sb = const.tile([6, H], F32)
        nc.gpsimd.dma_start(ws_sb[:], ws[:])
        bt_sb = const.tile([H, 1], F32)
        nc.gpsimd.dma_start(bt_sb[:], btot[:])
        vt_sb = const.tile([H, 1], F32)
        nc.gpsimd.dma_start(vt_sb[:], vt[:])
        # Sliding-window weight: vt (fp16) at column 32 of a zero [H, 64] tile.
        # lhsT = vtwin[:, 32-j:64-j] puts vt at local column j, so an M=32
        # matmul deposits the dot product on psum partition (32a + j).
        vtwin = const.tile([H, 64], F16)
        nc.vector.memset(vtwin[:], 0.0)
        nc.vector.tensor_copy(vtwin[:, 32:33], vt_sb[:])

        d_sbs, e_sbs = [], []
        for b in range(BC):
            ut_sb = pre.tile([3, U], F32, tag="ut")
            nc.sync.dma_start(ut_sb[:], userT[b])
            sv_sb = pre.tile([6, S], F32, tag="sv")
            nc.gpsimd.dma_start(sv_sb[:], servT[b])

            d_ps = pps.tile([H, U], F32, tag="dps", bufs=2)
            nc.tensor.matmul(d_ps[:], wu_sb[:], ut_sb[:])
            d_sb = dpool.tile([H, U], F32, tag="d")
            nc.scalar.activation(d_sb[:], d_ps[:], AF.Identity, bias=bt_sb[:])

            e_ps = pps.tile([H, S], F32, tag="eps", bufs=2)
            nc.tensor.matmul(e_ps[:], ws_sb[:], sv_sb[:])
            e_sb = epool.tile([H, S], F16, tag="e")
            nc.vector.tensor_copy(e_sb[:], e_ps[:])
            d_sbs.append(d_sb)
            e_sbs.append(e_sb)

        # Graduated block sizes: small blocks at kernel start (shorten the
        # serial preadd ramp before the first tanh) and at the very end
        # (shorten the final matmul burst + epilogue tail).
        def block_schedule(b, t):
            if b == 0 and t == 0:
                return [8, 8, 16, 32, 64, 64, 64]
            if t == 0:
                return [64, 64, 64, 64]
            if b == BC - 1:
                return [64, 64, 64, 36, 16]
            return [64, 64, 64, 52]

        for b in range(BC):
            d_sb, e_sb = d_sbs[b], e_sbs[b]
            for t in range(2):
                u0 = t * 256
                nu = min(U, u0 + 256) - u0  # 256 or 244
                R = nu // 2                 # psum rows used (128 or 122)
                ps = mps.tile([128, 512], F32, tag="ps")

                sched = block_schedule(b, t)
                assert sum(sched) == nu, (b, t, sched, nu)
                ub = u0
                for gu in sched:
                    X = xpool.tile([H, gu * S], F16, tag="X")
                    for j in range(gu):
                        nc.vector.tensor_scalar_add(
                            X[:, j * S:(j + 1) * S], e_sb[:],
                            d_sb[:, ub + j:ub + j + 1])
                    T = tpool.tile([H, gu * S], F16, tag="T")
                    nc.scalar.activation(T[:], X[:], AF.Tanh)
                    for p in range(gu // 2):
                        r = (ub - u0) // 2 + p
                        a, j = divmod(r, 32)
                        last = min(a * 32 + 31, R - 1)
                        nc.tensor.matmul(
                            ps[a * 32:a * 32 + 32, :],
                            vtwin[:, 32 - j:64 - j],
                            T[:, p * 512:(p + 1) * 512],
                            start=(j == 0),
                            stop=(r == last),
                            tile_position=(0, a * 32),
                        )
                    ub += gu
                assert ub == u0 + nu

                # --- masked softmax over s (free dim); 2 u per psum row ---
                mk8 = mpool.tile([128, 512], U8, tag="mk8")
                nc.sync.dma_start(
                    mk8[:R],
                    masks[b][u0:u0 + nu, :].rearrange("(r two) s -> r (two s)", two=2))
                mk16 = mpool.tile([128, 512], F16, tag="mk16")
                nc.vector.tensor_copy(mk16[:R], mk8[:R])

                mx = stpool.tile([128, 2], F32, tag="mx")
                nc.vector.reduce_max(
                    mx[:R], ps[:R].rearrange("r (two s) -> r two s", two=2), axis=AX.X)
                ngm = stpool.tile([128, 2], F32, tag="ngm")
                nc.vector.tensor_scalar_mul(ngm[:R], mx[:R], -10.0)

                eb = sxpool.tile([128, 512], F16, tag="eb")
                for hh in range(2):
                    nc.scalar.activation(
                        eb[:R, hh * 256:(hh + 1) * 256],
                        ps[:R, hh * 256:(hh + 1) * 256],
                        AF.Exp, bias=ngm[:R, hh:hh + 1], scale=10.0)
                em = sxpool.tile([128, 512], F16, tag="em")
                nc.vector.tensor_mul(em[:R], eb[:R], mk16[:R])

                sm = stpool.tile([128, 2], F32, tag="sm")
                nc.vector.reduce_sum(
                    sm[:R], em[:R].rearrange("r (two s) -> r two s", two=2), axis=AX.X)
                rc = stpool.tile([128, 2], F32, tag="rc")
                nc.vector.reciprocal(rc[:R], sm[:R])

                pr = prpool.tile([128, 512], F32, tag="pr")
                for hh in range(2):
                    nc.vector.tensor_scalar_mul(
                        pr[:R, hh * 256:(hh + 1) * 256],
                        em[:R, hh * 256:(hh + 1) * 256],
                        rc[:R, hh:hh + 1])
                # Split the store across DMA queues so the final tile's
                # writeback doesn't serialize on one queue at kernel end.
                dview = out[b][u0:u0 + nu, :].rearrange(
                    "(r two) s -> r (two s)", two=2)
                nq = 4
                step = (R + nq - 1) // nq
                for q in range(0, R, step):
                    hi = min(R, q + step)
                    nc.sync.dma_start(dview[q:hi], pr[q:hi])
    nc.compile()
    return nc


def _get_nc():
    if "nc" not in _CACHE:
        _CACHE["nc"] = _build_nc()
    return _CACHE["nc"]


def _prep_inputs(user, serv, mk, Wu, bu, Ws, bs, W1, W2, vt):
    wu_eff = np.ascontiguousarray((Wu @ W2).astype(np.float32))
    ws_eff = np.ascontiguousarray((Ws[:6] @ W1).astype(np.float32))
    btot = np.ascontiguousarray((bu @ W2 + bs @ W1).astype(np.float32).reshape(H, 1))
    vtc = np.ascontiguousarray(vt.astype(np.float32).reshape(H, 1))
    userT = np.ascontiguousarray(user[:, :, :3].transpose(0, 2, 1).astype(np.float32))
    servT = np.ascontiguousarray(serv.transpose(0, 2, 1).astype(np.float32))
    mku8 = np.ascontiguousarray(mk.astype(np.uint8))
    in_maps = []
    for c in range(N_CORES):
        sl = slice(c * BC, (c + 1) * BC)
        in_maps.append({
            "userT": np.ascontiguousarray(userT[sl]),
            "servT": np.ascontiguousarray(servT[sl]),
            "masks": np.ascontiguousarray(mku8[sl]),
            "wu_eff": wu_eff,
            "ws_eff": ws_eff,
            "btot": btot,
            "vt": vtc,
        })
    return in_maps


def kernel(user_input_seq_with_stay, server_input_seq, masks,
           Wu, bu, Ws, bs, W1, W2, vt, _trace=False):
    user = np.asarray(user_input_seq_with_stay, np.float32)
    serv = np.asarray(server_input_seq, np.float32)
    mk = np.asarray(masks)
    Wu = np.asarray(Wu, np.float32)
    bu = np.asarray(bu, np.float32)
    Ws = np.asarray(Ws, np.float32)
    bs = np.asarray(bs, np.float32)
    W1 = np.asarray(W1, np.float32)
    W2 = np.asarray(W2, np.float32)
    vt = np.asarray(vt, np.float32)

    in_maps = _prep_inputs(user, serv, mk, Wu, bu, Ws, bs, W1, W2, vt)
    nc = _get_nc()
    res = run_bass_kernel_spmd(nc, in_maps, list(range(N_CORES)), trace=_trace)
    _CACHE["last"] = res
    return np.concatenate(
        [res.results[c]["probs"] for c in range(N_CORES)], axis=0)
